# revision 9
# baseline (speedup 1.0000x reference)
"""Causal self-attention (B=2, T=2048, C=1024, H=16, Dh=64) on 8 trn2 NeuronCores.

Sharding: 2-way data-parallel over batch x 4-way tensor-parallel over heads.
Core c handles batch b=c//4 and heads 4g..4g+3 where g=c%4; it returns a
partial [T, C] row-projection which the host sums over the 4 head groups.

v2 schedule (vs the 207us baseline):
- DMA-paced start: x and the qkv weights stream on two rings while the
  first qk projection wave runs k-OUTER across 8 PSUM banks, so the PE
  starts ~1.5us in instead of waiting for the full 4MB of x.
- Chunk-major attention: outer loop over q-chunks, then heads, then
  k-tiles. Only one PV accumulator bank is live at a time, leaving PSUM
  room to double-buffer scores, and the output projection (+ HBM
  writeback) for chunk j fires as soon as its 4 heads finish - output DMA
  is spread across the whole attention span instead of a 13us tail.
- No mask matmuls: the causal triangle of each diagonal block is zeroed
  POST-exp by a gpsimd affine_select (idle engine), saving ~20k PE
  columns and 64 weight loads.
- PV lags scores by one k-tile so the scalar-engine exp latency never
  stalls the PE; independent GEMM work (V projection, second qk wave,
  output projection) is interleaved into the attention loops as filler.
- Epilogue per (chunk, head): softmax denominator comes free as a ones
  column in the PV stationary; 1/l via reciprocal_approx_fast (DVE) +
  gpsimd partition broadcast, fused into the PSUM->SBUF normalize mul.
"""

import numpy as np
import ml_dtypes
from contextlib import ExitStack

import concourse.bass as bass
import concourse.tile as tile
from concourse import bacc, mybir, bass_utils

F32 = mybir.dt.float32
BF16 = mybir.dt.bfloat16

T = 2048
C = 1024
HL = 4   # local heads per core
DH = 64
NKT = T // 128   # 16 k-tiles
NQ = T // 512    # 4 q-chunks
NCC = C // 128   # 8 contraction chunks


def _pin_act_table():
    import concourse.bacc as bacc_mod
    from concourse.hw_specs import get_activation_tables as real

    def only_combined(arch):
        t = real(arch)
        name = "natural_log_exp_and_others"
        if name in t:
            return {name: t[name]}
        return t

    bacc_mod.get_activation_tables = only_combined


def build_nc():
    _pin_act_table()
    nc = bacc.Bacc("TRN2", target_bir_lowering=False, debug=False)
    xt_d = nc.dram_tensor("xt", [C, T], BF16, kind="ExternalInput").ap()
    wqkt_d = nc.dram_tensor("wqkt", [C, 512], BF16, kind="ExternalInput").ap()
    wvt_d = nc.dram_tensor("wvt", [C, 256], BF16, kind="ExternalInput").ap()
    wpt_d = nc.dram_tensor("wpt", [256, C], BF16, kind="ExternalInput").ap()
    p_d = nc.dram_tensor("p", [T, C], F32, kind="ExternalOutput").ap()

    with tile.TileContext(nc) as tc:
        with ExitStack() as ctx:
            _body(ctx, tc, xt_d, wqkt_d, wvt_d, wpt_d, p_d)
    nc.compile()
    return nc


def _body(ctx, tc, xt_d, wqkt_d, wvt_d, wpt_d, p_d):
    nc = tc.nc
    Exp = mybir.ActivationFunctionType.Exp

    persist = ctx.enter_context(tc.tile_pool(name="persist", bufs=1))
    ptp = ctx.enter_context(tc.tile_pool(name="ptp", bufs=6))
    rrp = ctx.enter_context(tc.tile_pool(name="rrp", bufs=4))
    tmpn = ctx.enter_context(tc.tile_pool(name="tmpn", bufs=2))
    pout = ctx.enter_context(tc.tile_pool(name="pout", bufs=3))
    # PSUM (8 banks of [128,512]f32): tag "wa" ring of 4 (qk waves, V,
    # proj filler units), "st" ring of 2 (score stagger), "ot" ring of 2
    # (PV accumulators). 4+2+2 = 8.
    pp = ctx.enter_context(tc.tile_pool(name="pp", bufs=1, space="PSUM"))

    # ---- persistent SBUF tiles ----
    onesf = persist.tile([128, 64], F32, tag="onesf")
    xT = persist.tile([128, NCC, T], BF16, tag="xT")
    wqkT = persist.tile([128, NCC, 512], BF16, tag="wqkT")
    wvT = persist.tile([128, NCC, 256], BF16, tag="wvT")
    wpT = persist.tile([128, 2, C], BF16, tag="wpT")
    qkT = [persist.tile([128, T], BF16, tag=f"qkT{m}", name=f"qkT{m}")
           for m in range(4)]
    vs = [persist.tile([128, HL, 128], BF16, tag=f"vs{i}", name=f"vs{i}")
          for i in range(NKT)]
    otj = [persist.tile([128, 2, 512], BF16, tag=f"otj{j}", name=f"otj{j}")
           for j in range(NQ)]

    nc.gpsimd.memset(onesf[:], 1.0)
    for i in range(NKT):
        # col 64 of each head strip = softmax denominator ones; 65..127 pad.
        nc.vector.tensor_copy(
            vs[i][:, :, 64:128],
            onesf[:, 0:64].rearrange("p (a b) -> p a b", a=1).to_broadcast(
                (128, HL, 64)))

    # ---- input DMA: x on the sync ring, weights on the gpsimd ring ----
    for k in range(NCC):
        nc.sync.dma_start(xT[:, k, :], xt_d[k * 128:(k + 1) * 128, :])
    for k in range(NCC):
        nc.gpsimd.dma_start(wqkT[:, k, :], wqkt_d[k * 128:(k + 1) * 128, :])
    for k in range(NCC):
        nc.gpsimd.dma_start(wvT[:, k, :], wvt_d[k * 128:(k + 1) * 128, :])
    for c in range(2):
        nc.gpsimd.dma_start(wpT[:, c, :], wpt_d[c * 128:(c + 1) * 128, :])

    # ---- qk waves m=0 then m=2 (heads 0,1), k-outer, DMA-paced ----
    for m in (0, 2):
        psA = [pp.tile([128, 512], F32, tag="wa", bufs=4, name=f"wa{m}_{n}")
               for n in range(NQ)]
        for k in range(NCC):
            for n in range(NQ):
                nc.tensor.matmul(
                    psA[n][:],
                    lhsT=wqkT[:, k, m * 128:(m + 1) * 128],
                    rhs=xT[:, k, n * 512:(n + 1) * 512],
                    start=(k == 0), stop=(k == NCC - 1))
        for n in range(NQ):
            if n % 2 == 0:
                nc.scalar.copy(qkT[m][:, n * 512:(n + 1) * 512], psA[n][:])
            else:
                nc.vector.tensor_copy(qkT[m][:, n * 512:(n + 1) * 512],
                                      psA[n][:])

    # ---- filler units (emitted between attention blocks) ----
    def emit_A2_unit(m, n):  # one (m,n) block of the second qk wave
        ps = pp.tile([128, 512], F32, tag="wa", bufs=4)
        for k in range(NCC):
            nc.tensor.matmul(
                ps[:],
                lhsT=wqkT[:, k, m * 128:(m + 1) * 128],
                rhs=xT[:, k, n * 512:(n + 1) * 512],
                start=(k == 0), stop=(k == NCC - 1))
        if n % 2 == 0:
            nc.scalar.copy(qkT[m][:, n * 512:(n + 1) * 512], ps[:])
        else:
            nc.vector.tensor_copy(qkT[m][:, n * 512:(n + 1) * 512], ps[:])

    def emit_V_unit(i):  # v tile i
        ps = pp.tile([128, 256], F32, tag="wa", bufs=4)
        for k in range(NCC):
            nc.tensor.matmul(
                ps[:],
                lhsT=xT[:, k, i * 128:(i + 1) * 128],
                rhs=wvT[:, k, :],
                start=(k == 0), stop=(k == NCC - 1))
        nc.vector.tensor_copy(
            vs[i][:, :, 0:64], ps[:].rearrange("p (h d) -> p h d", h=HL))

    def emit_proj(j):  # output projection for q-chunk j + writeback
        for tbl in range(4):
            po = pout.tile([128, C], F32, tag="po")
            for n2 in range(2):
                ps = pp.tile([128, 512], F32, tag="wa", bufs=4)
                for c in range(2):
                    nc.tensor.matmul(
                        ps[:],
                        lhsT=otj[j][:, c, tbl * 128:(tbl + 1) * 128],
                        rhs=wpT[:, c, n2 * 512:(n2 + 1) * 512],
                        start=(c == 0), stop=(c == 1))
                nc.any.tensor_copy(po[:, n2 * 512:(n2 + 1) * 512], ps[:])
            tb = 4 * j + tbl
            nc.sync.dma_start(p_d[tb * 128:(tb + 1) * 128, :], po[:])

    # ---- attention: chunk-major ----
    def emit_chunk_head(j, h, fillers):
        """Chunk j of head h: k-tiles 0..4j+3, PV staggered one tile back.
        `fillers` = list of zero-arg emit fns to interleave for PE slack."""
        part = (h % 2) * 64
        qt = qkT[h // 2]
        kt = qkT[2 + h // 2]
        nkt = 4 * j + 4  # k-tiles in this chunk
        otp = pp.tile([128, 512], F32, tag="ot", bufs=2, name=f"ot{j}_{h}")
        pend = None  # (i, pt, co) awaiting PV
        fq = list(fillers)

        def scores_tile(i):
            co = (i - 4 * j) * 128 if i >= 4 * j else 0
            stp = pp.tile([128, 512], F32, tag="st", bufs=2)
            nc.tensor.matmul(
                stp[:, co:512],
                lhsT=kt[part:part + 64, i * 128:(i + 1) * 128],
                rhs=qt[part:part + 64, j * 512 + co:(j + 1) * 512],
                start=True, stop=True)
            pt = ptp.tile([128, 512], BF16, tag="pt")
            nc.scalar.activation(pt[:, co:512], stp[:, co:512], Exp)
            if i >= 4 * j:  # diagonal: zero exp'd upper triangle on gpsimd
                nc.gpsimd.affine_select(
                    out=pt[:, co:co + 128], in_=pt[:, co:co + 128],
                    compare_op=mybir.AluOpType.is_ge,
                    fill=0.0, base=0, channel_multiplier=-1,
                    pattern=[[1, 128]])
            return pt, co

        def pv_tile(i, pt, co):
            nc.tensor.matmul(
                otp[:, co:512],
                lhsT=vs[i][:, h, :],
                rhs=pt[:, co:512],
                start=(i == 0), stop=(i == nkt - 1))

        for i in range(nkt):
            if fq:
                fq.pop(0)()
            pt, co = scores_tile(i)
            if pend is not None:
                pv_tile(*pend)
            pend = (i, pt, co)
        pv_tile(*pend)
        for f in fq:
            f()

        # epilogue: normalize by 1/l (l = ones-column row 64 of otp)
        ls = rrp.tile([1, 512], F32, tag="ls")
        nc.vector.tensor_copy(ls[:], otp[64:65, :])
        li = rrp.tile([1, 512], F32, tag="li")
        nc.vector.reciprocal_approx_fast(li[:], ls[:])
        lb = rrp.tile([64, 512], F32, tag="lb")
        nc.gpsimd.partition_broadcast(lb[:], li[:])
        if h % 2 == 0:
            nc.vector.tensor_mul(otj[j][0:64, h // 2, :], otp[0:64, :], lb[:])
        else:
            tm = tmpn.tile([64, 512], BF16, tag="tm")
            nc.vector.tensor_mul(tm[:], otp[0:64, :], lb[:])
            nc.gpsimd.dma_start(otj[j][64:128, h // 2, :], tm[:])

    # filler inventory: V tiles 0..15, A2 blocks (m=1,3 x n=0..3).
    # Chunk j needs vs[0..4j+3] and heads 2,3 of chunk j need qk wave A2.
    # Pre-attention: A2 entirely (heads 2,3 of chunk 0 need it) + vs[0..3].
    for m in (1, 3):
        for n in range(NQ):
            emit_A2_unit(m, n)
    for i in range(4):
        emit_V_unit(i)

    # remaining V tiles: chunk 1 needs vs[4..7] -> emit inside chunk 0;
    # vs[8..11] inside chunk 1; vs[12..15] inside chunk 2.
    vnext = 4

    def v_fillers(j):
        nonlocal vnext
        need = min(NKT, 4 * (j + 1) + 4 + 4)  # stay a chunk ahead
        fs = []
        while vnext < need:
            i = vnext
            fs.append(lambda i=i: emit_V_unit(i))
            vnext += 1
        return fs

    for j in range(NQ):
        fs = v_fillers(j)
        per_head = [fs[k::HL] for k in range(HL)]
        for h in range(HL):
            emit_chunk_head(j, h, per_head[h])
        emit_proj(j)


_NC_CACHE = None


def _get_nc():
    global _NC_CACHE
    if _NC_CACHE is None:
        _NC_CACHE = build_nc()
    return _NC_CACHE


def make_in_maps(x, w_qkv, w_proj):
    x = np.asarray(x, np.float32)
    w_qkv = np.asarray(w_qkv, np.float32)
    w_proj = np.asarray(w_proj, np.float32)
    bf = ml_dtypes.bfloat16
    in_maps = []
    for c in range(8):
        b, g = divmod(c, 4)
        wq = w_qkv[g * 256:(g + 1) * 256] * 0.125  # fold 1/sqrt(Dh)
        wk = w_qkv[C + g * 256:C + (g + 1) * 256]
        wv = w_qkv[2 * C + g * 256:2 * C + (g + 1) * 256]
        wqk = np.concatenate([wq, wk], 0)  # [512, C]
        in_maps.append({
            "xt": np.ascontiguousarray(x[b].T).astype(bf),
            "wqkt": np.ascontiguousarray(wqk.T).astype(bf),
            "wvt": np.ascontiguousarray(wv.T).astype(bf),
            "wpt": np.ascontiguousarray(
                w_proj[:, g * 256:(g + 1) * 256].T).astype(bf),
        })
    return in_maps


def combine(results):
    return np.stack(
        [results[4 * b]["p"] + results[4 * b + 1]["p"]
         + results[4 * b + 2]["p"] + results[4 * b + 3]["p"]
         for b in range(2)], 0)


def kernel(x, w_qkv, w_proj):
    nc = _get_nc()
    res = bass_utils.run_bass_kernel_spmd(
        nc, make_in_maps(x, w_qkv, w_proj), core_ids=list(range(8)))
    return combine(res.results)
